# revision 20
# baseline (speedup 1.0000x reference)
"""Trainium2 Bass kernel for nn_Depth_CA (depth-coded-aperture Wiener pipeline).

Strategy
--------
Every fft/ifft+shift combo in the reference is a constant 256x256 complex
matrix sandwich Y = A @ X @ A.T computed on the PE array as two matmul
groups with the DATA stationary and host-packed constants moving; PSUM
accumulation implements the complex arithmetic.

Math shortcuts vs the reference:
  * mid-pipeline result/max(result) cancels against the final normalize.
  * psf is real, so psf_ifr == conj(psffr)/65536: the whole ifft2 sandwich
    for the Wiener numerator is replaced by elementwise work
    (K = conj(psffr) / (65536*(|psffr|^2 + param))).
  * blur/Wiener inverse transforms are mathematically real: second matmul
    groups compute the real part only.

Performance structure (vs the ~900us first version):
  * blur+wiener elementwise/matmul data in bf16 (2x DVE tensor_tensor,
    same PE stream rate); PSF synthesis stays f32r.
  * kker / mag2 / mapt SBUF-resident (no DRAM roundtrips).
  * software pipelining by emission order: stage1 is band-major with
    blur(b=0) interleaved; then blur(b)/AllReduce(b)/wiener(b-1) rotate so
    collectives hide under compute.
  * final normalize fans out across scalar/vector/gpsimd + 4 DMA queues.
  * reciprocal_approx_fast for Wiener denominators; fused tensor_scalar.

Sharding: depths padded 15->16, 2 per core across 8 cores; image-space
work (imgft, result FFT) replicated (collective bw makes sharding it a
loss); per-batch AllReduce(add) for the depth-summed result and one
AllReduce(max) for the final normalization.
"""
import os
import sys

for _p in ("/opt/trn_rl_repo", os.path.expanduser("~/.axon_site/_ro/trn_rl_repo")):
    if os.path.isdir(_p) and _p not in sys.path:
        sys.path.insert(0, _p)

import numpy as np

N = 256
ND, NB, B = 15, 3, 4
NDP = 16               # padded depth count
NCORES = 8
DPC = NDP // NCORES    # depths per core = 2

BF16 = True            # blur+wiener datapath dtype toggle

# ---------------------------------------------------------------- host constants
def _host_constants():
    ZI, Z0, RADII, PX = 0.05, 2.5, 0.002, 6.22e-6
    F_ = 1.0 / (1.0 / ZI + 1.0 / Z0)
    L_SEN = PX * N
    L_LEN = 2 * RADII * 2
    LAMB = np.array([460.0, 550.0, 640.0]) * 1e-9

    def deta(l_um):
        l = np.asarray(l_um, dtype=np.float64)
        return (1.5375 + 0.00829045 * l**-2 - 0.000211046 * l**-4) - 1.0

    R_ = F_ * deta(5.5e-7 * 1e6)
    FLMB = R_ / deta(LAMB * 1e6)
    ZS = np.sort(-3 * np.log(np.linspace(0.9, 11, ND)) + 8)
    DU = L_LEN / N
    u = np.arange(-L_LEN / 2, L_LEN / 2, DU)
    X_, Y_ = np.meshgrid(u, u)
    XY = X_ * X_ + Y_ * Y_
    RAD = (np.sqrt(XY) <= RADII).astype(np.float64)
    fx1 = np.fft.fftshift(np.arange(-1 / (2 * DU), 1 / (2 * DU), 1 / L_LEN))
    FX1, FY1 = np.meshgrid(fx1, fx1)
    FF = FX1 * FX1 + FY1 * FY1

    K_ = 2 * np.pi / LAMB
    COEF = (-K_ / (2 * FLMB[0]))[None, :] + K_[None, :] / (2 * ZS[:, None]) \
        + (np.pi * (L_LEN - L_SEN) / (LAMB * ZI * L_LEN))[None, :]
    PHASE1 = (COEF[:, :, None, None] * XY[None, None]).astype(np.float32)
    PHASE2 = ((np.pi * LAMB * ZI * L_LEN / L_SEN)[None, :, None, None]
              * FF[None, None]).astype(np.float32)

    W1 = RAD[None, None] * np.exp(1j * PHASE1.astype(np.float64))    # (15,3,N,N)
    W2 = np.exp(-1j * PHASE2.astype(np.float64)[0])                  # (3,N,N)

    j = np.arange(N)
    F = np.exp(-2j * np.pi * np.outer(j, j) / N)
    G = np.conj(F) / N
    P = np.zeros((N, N))
    P[j, (j + N // 2) % N] = 1.0
    A1 = F @ P
    A2 = P @ G
    Fc = P @ F @ P
    Gc = P @ G @ P
    return W1, W2, (A1, A2, Fc, Gc)


def _pack_field(X):
    """complex (N,N) -> float32 [2, 128, 512] = per row-block [Re | Im]."""
    out = np.empty((2, 128, 512), np.float32)
    for rb in range(2):
        out[rb, :, 0:256] = X.real[rb * 128:(rb + 1) * 128, :]
        out[rb, :, 256:512] = X.imag[rb * 128:(rb + 1) * 128, :]
    return out


def _pack_moving(A):
    """constant A -> float32 [2 variants, 2 k-chunks, 128, 512] moving ops."""
    AT = A.T.copy()
    out = np.empty((2, 2, 128, 512), np.float32)
    for k in range(2):
        r = AT.real[k * 128:(k + 1) * 128, :]
        i = AT.imag[k * 128:(k + 1) * 128, :]
        out[0, k, :, 0:256] = r
        out[0, k, :, 256:512] = i
        out[1, k, :, 0:256] = -i
        out[1, k, :, 256:512] = r
    return out


_CONST_CACHE = {}


def _get_device_arrays():
    """Host constants packed into the device DMA layouts."""
    if "dev" not in _CONST_CACHE:
        W1, W2, mats = _host_constants()
        # moving constants [128, n*2048]: col = ((a*2+v)*2+k)*512 + n
        f32_mats = mats[:3] if BF16 else mats  # Gc only used via bf16 table
        movA = np.concatenate(
            [_pack_moving(A).reshape(4, 128, 512).transpose(1, 0, 2).reshape(128, 2048)
             for A in f32_mats], axis=1)
        # bf16 copy of the Gc moving table [128, 2048] for the blur/wiener MMs
        movGc = _pack_moving(mats[3]).reshape(4, 128, 512).transpose(1, 0, 2).reshape(128, 2048).copy()
        # w2 [128, 3072]: col = (c*2+rb)*512 + n  (per-rb [Re|Im])
        w2p = np.concatenate(
            [_pack_field(W2[c]).transpose(1, 0, 2).reshape(128, 1024)
             for c in range(NB)], axis=1)
        # w1 table [48, 128, 1024] d-major over padded depths
        w1rows = []
        for d in range(NDP):
            dd = d if d < ND else 0
            for c in range(NB):
                w1rows.append(_pack_field(W1[dd, c]).transpose(1, 0, 2).reshape(128, 1024))
        w1all = np.stack(w1rows)
        R = np.kron(np.eye(16), np.ones((1, 16))).astype(np.float32)
        _CONST_CACHE["dev"] = (np.ascontiguousarray(movA), np.ascontiguousarray(movGc),
                               np.ascontiguousarray(w2p), np.ascontiguousarray(w1all), R)
    return _CONST_CACHE["dev"]


# ---------------------------------------------------------------- device program
_REPS = int(os.environ.get("BASS_KERNEL_REPS", "1"))

A1_I, A2_I, FC_I, GC_I = 0, 1, 2, 3


def _build_program():
    host_arrays = _get_device_arrays()
    reps = _REPS
    import concourse.bass as bass
    import concourse.bass_isa as bass_isa
    import concourse.bacc as bacc
    import concourse.mybir as mybir
    import concourse.tile as tile

    dt = mybir.dt
    ALU = mybir.AluOpType
    ACTF = mybir.ActivationFunctionType
    DT16 = dt.bfloat16 if BF16 else dt.float32r

    movA_h, movGc_h, w2_h, w1all_h, R_h = host_arrays

    nc = bacc.Bacc("TRN2", target_bir_lowering=False, debug=False,
                   num_devices=NCORES)

    def inline(data, name, f32r=False):
        h = nc.inline_tensor(np.ascontiguousarray(data), name=name)
        if f32r:
            mls = nc.lookup_mls(h)
            mls.dtype = dt.float32r
            h = bass.DRamTensorHandle(name, list(data.shape), dt.float32r)
        return h.ap()

    NMOV = movA_h.shape[1]
    movA_d = inline(movA_h, "mova", f32r=True)                 # [128, NMOV]
    w2_d = inline(w2_h, "w2")                                  # [128, 3072]
    w1all_d = inline(w1all_h, "w1all")                         # [48, 128, 1024]
    r_d = inline(R_h, "rmat")                                  # [16, 256]
    if BF16:
        import ml_dtypes
        movGc16_h = movGc_h.astype(ml_dtypes.bfloat16)
        movGc16_d = inline(movGc16_h, "movgc16")               # [128, 2048] bf16

    img_d = nc.dram_tensor("imgf", [128, 6144], dt.float32r, kind="ExternalInput").ap()
    map_d = nc.dram_tensor("mapf", [B, 128, DPC * 512], dt.float32, kind="ExternalInput").ap()
    ht_d = nc.dram_tensor("ht", [16, 16], dt.float32, kind="ExternalInput").ap()
    par_d = nc.dram_tensor("param", [1, 1], dt.float32, kind="ExternalInput").ap()
    mask_d = nc.dram_tensor("mask", [1, DPC], dt.float32, kind="ExternalInput").ap()
    out_d = nc.dram_tensor("out_recov", [DPC, NB, B, 128, 512], dt.float32, kind="ExternalOutput").ap()

    with tile.TileContext(nc) as tc:
        with (
            tc.tile_pool(name="res", bufs=1) as res,
            tc.tile_pool(name="wk", bufs=2) as wk,
            tc.tile_pool(name="ps", bufs=4, space="PSUM") as ps,
            tc.tile_pool(name="dram", bufs=1, space="DRAM") as dram,
        ):
            # ---------------- resident constants; tiny CA inputs first so the
            # CA matmuls are not stuck behind megabyte constant loads
            ht_t = res.tile([16, 16], dt.float32, tag="ht_t", name="ht_t")
            r_t = res.tile([16, 256], dt.float32, tag="r_t", name="r_t")
            nc.sync.dma_start(ht_t[:], ht_d[:])
            nc.sync.dma_start(r_t[:], r_d[:])
            movall = res.tile([128, NMOV], dt.float32r, tag="movall", name="movall")
            # per-matrix chunks, first-needed first (Fc feeds the first imgft)
            nc.sync.dma_start(movall[:, FC_I * 2048:FC_I * 2048 + 2048],
                              movA_d[:, FC_I * 2048:FC_I * 2048 + 2048])
            nc.sync.dma_start(movall[:, A1_I * 2048:A1_I * 2048 + 2048],
                              movA_d[:, A1_I * 2048:A1_I * 2048 + 2048])
            nc.sync.dma_start(movall[:, A2_I * 2048:A2_I * 2048 + 2048],
                              movA_d[:, A2_I * 2048:A2_I * 2048 + 2048])

            def mov(a, v, k):
                o = ((a * 2 + v) * 2 + k) * 512
                return movall[:, o:o + 512]

            if BF16:
                movgc16 = res.tile([128, 2048], dt.bfloat16, tag="movgc16", name="movgc16")
                nc.sync.dma_start(movgc16[:], movGc16_d[:])

                def mov16(v, k):
                    o = (v * 2 + k) * 512
                    return movgc16[:, o:o + 512]
            else:
                def mov16(v, k):
                    return mov(GC_I, v, k)

            w2all = res.tile([128, 3072], dt.float32, tag="w2all", name="w2all")
            nc.sync.dma_start(w2all[:], w2_d[:])

            def w2v(c, rb):
                o = (c * 2 + rb) * 512
                return w2all[:, o:o + 512]

            par1 = res.tile([1, 1], dt.float32, tag="par1", name="par1")
            nc.sync.dma_start(par1[:], par_d[:])
            par128 = res.tile([128, 1], dt.float32, tag="par128", name="par128")
            nc.gpsimd.partition_broadcast(par128[:], par1[:])
            mask1 = res.tile([1, DPC], dt.float32, tag="mask1", name="mask1")
            nc.sync.dma_start(mask1[:], mask_d[:])
            mask128 = res.tile([128, DPC], dt.float32, tag="mask128", name="mask128")
            nc.gpsimd.partition_broadcast(mask128[:], mask1[:])

            # resident Map fields: [128, DPC*512] per batch
            mapt = [res.tile([128, DPC * 512], dt.float32, tag=f"mapt{b}", name=f"mapt{b}")
                    for b in range(B)]
            for b in range(B):
                nc.scalar.dma_start(mapt[b][:], map_d[b])

            # ---------------- CA = R^T @ (H @ R)  (plain fp32)
            ca_mid_ps = ps.tile([16, 256], dt.float32, tag="psB", bufs=4, name="ca_mid_ps")
            nc.tensor.matmul(ca_mid_ps[:], ht_t[:], r_t[:], start=True, stop=True)
            ca_mid = res.tile([16, 256], dt.float32, tag="ca_mid", name="ca_mid")
            nc.vector.tensor_copy(ca_mid[:], ca_mid_ps[:])
            ca = [res.tile([128, 256], dt.float32, tag=f"ca{mb}", name=f"ca{mb}")
                  for mb in range(2)]
            for mb in range(2):
                ca_ps = ps.tile([128, 256], dt.float32, tag="psB", bufs=4, name=f"ca_ps{mb}")
                nc.tensor.matmul(ca_ps[:], r_t[:, mb * 128:(mb + 1) * 128],
                                 ca_mid[:], start=True, stop=True)
                nc.vector.tensor_copy(ca[mb][:], ca_ps[:])

            ones65536 = res.tile([128, 128], dt.float32, tag="ones65536", name="ones65536")
            nc.vector.memset(ones65536[:], 65536.0)

            # ---------------- helpers
            MM1_NAMES = ("s1a", "s1c", "pfa", "ifa", "bla", "rfa", "wna")

            def mm_sandwich_half(stat, a_idx, is_complex, name, m16=False):
                """PSUM[mb] = S^T @ A^T.  `stat` = list of 2 per-k-chunk APs:
                complex: [128,512] ([Re|Im]); real: [128,256]."""
                ptag = "psA" if name in MM1_NAMES else "psB"
                mv = (lambda v, k: mov16(v, k)) if m16 else (lambda v, k: mov(a_idx, v, k))
                psums = []
                for mb in range(2):
                    acc = ps.tile([128, 512], dt.float32, tag=ptag, bufs=4, name=f"{name}_ps{mb}")
                    mms = []
                    for k in range(2):
                        mms.append((stat[k][:, mb * 128:(mb + 1) * 128], mv(0, k)))
                        if is_complex:
                            mms.append((stat[k][:, 256 + mb * 128:256 + (mb + 1) * 128],
                                        mv(1, k)))
                    for i, (lhsT, rhs) in enumerate(mms):
                        nc.tensor.matmul(acc[:], lhsT, rhs,
                                         start=(i == 0), stop=(i == len(mms) - 1))
                    psums.append(acc)
                return psums

            def mm_sandwich_real_out(stat, a_idx, name, m16=False):
                """Re-only PSUM[mb][128,256] = Re(S^T @ A^T), S complex packed."""
                mv = (lambda v, k: mov16(v, k)) if m16 else (lambda v, k: mov(a_idx, v, k))
                psums = []
                for mb in range(2):
                    acc = ps.tile([128, 256], dt.float32, tag="psB", bufs=4, name=f"{name}_ps{mb}")
                    mms = []
                    for k in range(2):
                        mms.append((stat[k][:, mb * 128:(mb + 1) * 128],
                                    mv(0, k)[:, 0:256]))
                        mms.append((stat[k][:, 256 + mb * 128:256 + (mb + 1) * 128],
                                    mv(1, k)[:, 0:256]))
                    for i, (lhsT, rhs) in enumerate(mms):
                        nc.tensor.matmul(acc[:], lhsT, rhs,
                                         start=(i == 0), stop=(i == len(mms) - 1))
                    psums.append(acc)
                return psums

            def drain(psums, name, dtype=dt.float32r):
                """PSUM pair -> SBUF pair, split across scalar+vector."""
                dtag, dbufs = ("drB", 6) if name in ("blu", "wnu") else ("drA", 5)
                out = [wk.tile([128, 512], dtype, tag=dtag, bufs=dbufs, name=f"{name}{mb}")
                       for mb in range(2)]
                nc.scalar.copy(out[0][:], psums[0][:])
                nc.vector.tensor_copy(out[1][:], psums[1][:])
                return out

            def cmul(out_rb, x_rb, y_rb, dtp, eng2=None):
                """one-rb complex mult: out [128,512] = x * y ([Re|Im] packed).
                eng2: engine for 2 of the 4 product ops (default vector)."""
                xr, xi = x_rb[:, 0:256], x_rb[:, 256:512]
                yr, yi = y_rb[:, 0:256], y_rb[:, 256:512]
                e2 = eng2 or nc.vector
                t1 = wk.tile([128, 256], dtp, tag="cms", bufs=10, name="cmt1")
                t2 = wk.tile([128, 256], dtp, tag="cms", bufs=10, name="cmt2")
                t3 = wk.tile([128, 256], dtp, tag="cms", bufs=10, name="cmt3")
                t4 = wk.tile([128, 256], dtp, tag="cms", bufs=10, name="cmt4")
                nc.vector.tensor_tensor(t1[:], xr, yr, op=ALU.mult)
                e2.tensor_tensor(t2[:], xi, yi, op=ALU.mult)
                nc.vector.tensor_tensor(out_rb[:, 0:256], t1[:], t2[:], op=ALU.subtract)
                nc.vector.tensor_tensor(t3[:], xr, yi, op=ALU.mult)
                e2.tensor_tensor(t4[:], xi, yr, op=ALU.mult)
                nc.vector.tensor_tensor(out_rb[:, 256:512], t3[:], t4[:], op=ALU.add)

            # ---------------- resident per-unit products
            # psffr in blur dtype (bf16) for the cmuls; f32 copy is transient
            psffr16 = [res.tile([128, 512], DT16, tag=f"pf16_{i}", name=f"pf16_{i}")
                       for i in range(DPC * NB * 2)]
            kker = [res.tile([128, 1024], DT16, tag=f"kker{u}", name=f"kker{u}")
                    for u in range(DPC * NB)]
            mag2sb = [res.tile([128, 512], DT16, tag=f"mag2_{i}", name=f"mag2_{i}")
                      for i in range(DPC * NB * B)]
            rmax = [res.tile([128, 512], DT16, tag=f"rmax{dl}", name=f"rmax{dl}")
                    for dl in range(DPC)]

            imgft_dr = dram.tile([B * NB, 128, 1024], DT16, name="imgft_dr")

            pid6 = nc.gpsimd.partition_id() * (DPC * NB)

            def emit_imgft(f):
                imS = wk.tile([128, 512], dt.float32r, tag="imS", name="imS")
                nc.sync.dma_start(imS[:], img_d[:, f * 512:(f + 1) * 512])
                stat = [imS[:, 0:256], imS[:, 256:512]]
                iu1 = drain(mm_sandwich_half(stat, FC_I, False, "ifa"), "ifu")
                ip2 = mm_sandwich_half(iu1, FC_I, True, "ifb")
                imo = wk.tile([128, 1024], DT16, tag="cfld", bufs=3, name="imo")
                nc.scalar.copy(imo[:, 0:512], ip2[0][:])
                nc.vector.tensor_copy(imo[:, 512:1024], ip2[1][:])
                nc.scalar.dma_start(imgft_dr[f], imo[:])

            def emit_unit(u):
                """stage1 PSF unit: psf -> psffr -> K, all SBUF-resident out."""
                c = u % NB
                w1t = wk.tile([128, 1024], dt.float32, tag="w1t", name="w1t")
                nc.gpsimd.dma_start(w1t[:], w1all_d[bass.ds(pid6 + u, 1)])
                ph = wk.tile([128, 1024], dt.float32r, tag="ph", name="ph")
                for rb in range(2):
                    o = rb * 512
                    nc.vector.tensor_tensor(ph[:, o:o + 256], w1t[:, o:o + 256],
                                            ca[rb][:], op=ALU.mult)
                    nc.vector.tensor_tensor(ph[:, o + 256:o + 512], w1t[:, o + 256:o + 512],
                                            ca[rb][:], op=ALU.mult)
                phs = [ph[:, 0:512], ph[:, 512:1024]]
                u1 = drain(mm_sandwich_half(phs, A1_I, True, "s1a"), "s1u1")
                ps2 = mm_sandwich_half(u1, A1_I, True, "s1b")
                u2 = drain(ps2, "s1u2", dtype=dt.float32)
                vu2 = wk.tile([128, 1024], dt.float32r, tag="cprod", bufs=2, name="vu2")
                for rb in range(2):
                    cmul(vu2[:, rb * 512:(rb + 1) * 512], u2[rb], w2v(c, rb),
                         dt.float32)
                vus = [vu2[:, 0:512], vu2[:, 512:1024]]
                u3 = drain(mm_sandwich_half(vus, A2_I, True, "s1c"), "s1u3")
                ps4 = mm_sandwich_half(u3, A2_I, True, "s1d")
                # psf = |vu3|^2 normalized (real field, rb-packed [128,512])
                psfu = wk.tile([128, 512], dt.float32r, tag="psfu", name="psfu")
                for rb in range(2):
                    t1 = wk.tile([128, 256], dt.float32, tag="cms", bufs=10, name="sq1")
                    t2 = wk.tile([128, 256], dt.float32, tag="cms", bufs=10, name="sq2")
                    nc.scalar.activation(t1[:], ps4[rb][:, 0:256], ACTF.Square)
                    nc.scalar.activation(t2[:], ps4[rb][:, 256:512], ACTF.Square)
                    nc.vector.tensor_tensor(psfu[:, rb * 256:(rb + 1) * 256],
                                            t1[:], t2[:], op=ALU.add)
                sums = wk.tile([128, 1], dt.float32, tag="sums", name="sums")
                nc.vector.tensor_reduce(sums[:], psfu[:], axis=mybir.AxisListType.X, op=ALU.add)
                tot_ps = ps.tile([128, 1], dt.float32, tag="psB", bufs=4, name="tot_ps")
                nc.tensor.matmul(tot_ps[:], ones65536[:], sums[:], start=True, stop=True)
                inv65536 = wk.tile([128, 1], dt.float32, tag="inv65536", name="inv65536")
                nc.vector.reciprocal_approx_fast(inv65536[:], tot_ps[:])   # 1/(65536*sum)
                inv128 = wk.tile([128, 1], dt.float32, tag="inv128", name="inv128")
                nc.vector.tensor_scalar_mul(inv128[:], inv65536[:], 65536.0)  # 1/sum
                neg65536 = wk.tile([128, 1], dt.float32, tag="neg65536", name="neg65536")
                nc.vector.tensor_scalar_mul(neg65536[:], inv65536[:], -1.0)
                # psffr(normalized) = (Fc |vu|^2 Fc) * (1/sum): run the sandwich on
                # the unnormalized field and fold 1/sum into the drains/K math so
                # the normalize chain is off the critical path.
                psts = [psfu[:, 0:256], psfu[:, 256:512]]
                pu1 = drain(mm_sandwich_half(psts, FC_I, False, "pfa"), "pfu")
                pp2 = mm_sandwich_half(pu1, FC_I, True, "pfb")
                nc.scalar.activation(psffr16[u * 2][:], pp2[0][:], ACTF.Copy,
                                     scale=inv128[:])
                nc.vector.tensor_scalar_mul(psffr16[u * 2 + 1][:], pp2[1][:], inv128[:])
                # K' = conj(psffr) / (65536*(|psffr|^2 + param)); wiener output
                # runs through abs() so any global sign is irrelevant.
                # Reads the pfb PSUM directly (raw), folding the 1/sum scale in.
                for rb in range(2):
                    fr = pp2[rb][:, 0:256]
                    fi = pp2[rb][:, 256:512]
                    t1 = wk.tile([128, 256], dt.float32, tag="cms", bufs=10, name="ab1")
                    t2 = wk.tile([128, 256], dt.float32, tag="cms", bufs=10, name="ab2")
                    nc.scalar.activation(t1[:], fr, ACTF.Square, scale=inv128[:])
                    nc.scalar.activation(t2[:], fi, ACTF.Square, scale=inv128[:])
                    s12 = wk.tile([128, 256], dt.float32, tag="cms", bufs=10, name="s12")
                    nc.vector.tensor_tensor(s12[:], t1[:], t2[:], op=ALU.add)
                    dpos = wk.tile([128, 256], dt.float32, tag="cms", bufs=10, name="dpos")
                    nc.vector.tensor_scalar_add(dpos[:], s12[:], par128[:])
                    invp = wk.tile([128, 256], dt.float32, tag="cms", bufs=10, name="invp")
                    nc.vector.reciprocal_approx_fast(invp[:], dpos[:])
                    nc.vector.scalar_tensor_tensor(
                        kker[u][:, rb * 512:rb * 512 + 256], fr, inv65536[:], invp[:],
                        op0=ALU.mult, op1=ALU.mult)
                    nc.vector.scalar_tensor_tensor(
                        kker[u][:, rb * 512 + 256:rb * 512 + 512], fi, neg65536[:], invp[:],
                        op0=ALU.mult, op1=ALU.mult)

            def emit_blur(b, c, cc_in):
                f = b * NB + c
                imf = wk.tile([128, 1024], DT16, tag="cfld16", bufs=3, name="imf")
                nc.scalar.dma_start(imf[:], imgft_dr[f])
                racc = wk.tile([128, 512], dt.float32, tag="racc", bufs=2, name="racc")
                for dl in range(DPC):
                    u = dl * NB + c
                    bp = wk.tile([128, 1024], DT16, tag="cprod16", bufs=3, name="bp")
                    for rb in range(2):
                        cmul(bp[:, rb * 512:(rb + 1) * 512],
                             imf[:, rb * 512:(rb + 1) * 512], psffr16[u * 2 + rb][:],
                             DT16)
                    bps = [bp[:, 0:512], bp[:, 512:1024]]
                    bu1 = drain(mm_sandwich_half(bps, GC_I, True, "bla", m16=BF16),
                                "blu", dtype=DT16)
                    bp2 = mm_sandwich_real_out(bu1, GC_I, "blb", m16=BF16)
                    for rb in range(2):
                        mag = wk.tile([128, 256], dt.float32, tag="cms", bufs=10, name="mag")
                        nc.scalar.activation(mag[:], bp2[rb][:], ACTF.Abs)
                        mslc = mapt[b][:, (dl * 2 + rb) * 256:(dl * 2 + rb + 1) * 256]
                        if dl == 0:
                            nc.vector.tensor_tensor(racc[:, rb * 256:(rb + 1) * 256],
                                                    mag[:], mslc, op=ALU.mult)
                        else:
                            t2 = wk.tile([128, 256], dt.float32, tag="cms", bufs=10, name="bm2")
                            nc.vector.tensor_tensor(t2[:], mag[:], mslc, op=ALU.mult)
                            nc.vector.tensor_tensor(racc[:, rb * 256:(rb + 1) * 256],
                                                    racc[:, rb * 256:(rb + 1) * 256],
                                                    t2[:], op=ALU.add)
                nc.sync.dma_start(cc_in[b][c], racc[:])

            def emit_wiener(b, c, cc_out):
                rres = wk.tile([128, 512], dt.float32r, tag="rres", bufs=2, name="rres")
                nc.gpsimd.dma_start(rres[:], cc_out[b][c])
                rsts = [rres[:, 0:256], rres[:, 256:512]]
                ru1 = drain(mm_sandwich_half(rsts, FC_I, False, "rfa"), "rfu")
                rp2 = mm_sandwich_half(ru1, FC_I, True, "rfb")
                resfr = wk.tile([128, 1024], DT16, tag="cfld16", bufs=3, name="resfr")
                nc.scalar.copy(resfr[:, 0:512], rp2[0][:])
                nc.vector.tensor_copy(resfr[:, 512:1024], rp2[1][:])
                for dl in range(DPC):
                    u = dl * NB + c
                    wn = wk.tile([128, 1024], DT16, tag="cprod16", bufs=3, name="wn")
                    for rb in range(2):
                        cmul(wn[:, rb * 512:(rb + 1) * 512],
                             kker[u][:, rb * 512:(rb + 1) * 512],
                             resfr[:, rb * 512:(rb + 1) * 512], DT16)
                    wns = [wn[:, 0:512], wn[:, 512:1024]]
                    wu1 = drain(mm_sandwich_half(wns, GC_I, True, "wna", m16=BF16),
                                "wnu", dtype=DT16)
                    wp2 = mm_sandwich_real_out(wu1, GC_I, "wnb", m16=BF16)
                    mi = (dl * NB + c) * B + b
                    for rb in range(2):
                        nc.scalar.activation(mag2sb[mi][:, rb * 256:(rb + 1) * 256],
                                             wp2[rb][:], ACTF.Abs)
                    nc.vector.tensor_tensor(rmax[dl][:], rmax[dl][:], mag2sb[mi][:],
                                            op=ALU.max)

            for _rep in range(reps):
                cc_in = [dram.tile([NB, 128, 512], dt.float32, name=f"cc_in{b}_r{_rep}")
                         for b in range(B)]
                cc_out = [dram.tile([NB, 128, 512], dt.float32, name=f"cc_out{b}_r{_rep}",
                                    addr_space="Shared") for b in range(B)]
                ccm_in = dram.tile([1, 16], dt.float32, name=f"ccm_in_r{_rep}")
                ccm_out = dram.tile([1, 16], dt.float32, name=f"ccm_out_r{_rep}", addr_space="Shared")

                for dl in range(DPC):
                    nc.vector.memset(rmax[dl][:], 0.0)

                def emit_ar(b):
                    nc.gpsimd.collective_compute(
                        "AllReduce", ALU.add,
                        replica_groups=[list(range(NCORES))],
                        ins=[cc_in[b][:]], outs=[cc_out[b][:]],
                    )

                # ======== stage 1, band-major, with b=0 blur interleaved
                for c in range(NB):
                    emit_imgft(0 * NB + c)
                    emit_imgft(1 * NB + c)
                    emit_unit(0 * NB + c)          # dl = 0
                    emit_imgft(2 * NB + c)
                    emit_imgft(3 * NB + c)
                    emit_unit(1 * NB + c)          # dl = 1
                    emit_blur(0, c, cc_in)
                    emit_blur(1, c, cc_in)
                emit_ar(0)
                emit_ar(1)
                # ======== rotate blur(b) / AllReduce(b) / wiener(b-1)
                for c in range(NB):
                    emit_blur(2, c, cc_in)
                emit_ar(2)
                for c in range(NB):
                    emit_wiener(0, c, cc_out)
                for c in range(NB):
                    emit_blur(3, c, cc_in)
                emit_ar(3)
                for c in range(NB):
                    emit_wiener(1, c, cc_out)
                for c in range(NB):
                    emit_wiener(2, c, cc_out)
                for c in range(NB):
                    emit_wiener(3, c, cc_out)

                # ======== global max + final normalize
                r1f = wk.tile([128, 512], dt.float32, tag="fin", bufs=3, name="r1f")
                nc.vector.tensor_scalar_mul(r1f[:], rmax[1][:], mask128[:, 1:2])
                r0f = wk.tile([128, 512], dt.float32, tag="fin", bufs=3, name="r0f")
                nc.vector.tensor_copy(r0f[:], rmax[0][:])
                comb = wk.tile([128, 512], dt.float32, tag="fin", bufs=3, name="comb")
                nc.vector.tensor_tensor(comb[:], r0f[:], r1f[:], op=ALU.max)
                mx = wk.tile([128, 1], dt.float32, tag="mx", name="mx")
                nc.vector.tensor_reduce(mx[:], comb[:], axis=mybir.AxisListType.X, op=ALU.max)
                gmx128 = wk.tile([128, 1], dt.float32, tag="gmx128", name="gmx128")
                nc.gpsimd.partition_all_reduce(gmx128[:], mx[:], channels=128,
                                               reduce_op=bass_isa.ReduceOp.max)
                ones16 = wk.tile([1, 16], dt.float32, tag="ones16", name="ones16")
                nc.vector.memset(ones16[:], 1.0)
                gmx16 = wk.tile([1, 16], dt.float32, tag="gmx16", name="gmx16")
                nc.vector.tensor_scalar_mul(gmx16[:], ones16[:], gmx128[0:1, :])
                nc.sync.dma_start(ccm_in[:], gmx16[:])
                nc.gpsimd.collective_compute(
                    "AllReduce", ALU.max,
                    replica_groups=[list(range(NCORES))],
                    ins=[ccm_in[:]], outs=[ccm_out[:]],
                )
                gm = wk.tile([1, 1], dt.float32, tag="gm", name="gm")
                nc.sync.dma_start(gm[:], ccm_out[0:1, 0:1])
                ginv = wk.tile([1, 1], dt.float32, tag="ginv", name="ginv")
                nc.vector.reciprocal(ginv[:], gm[:])
                ginv128 = wk.tile([128, 1], dt.float32, tag="ginv128", name="ginv128")
                nc.gpsimd.partition_broadcast(ginv128[:], ginv[:])

                # fan the 48 scale+stores across three engines / four queues
                engs = [(nc.scalar, "s"), (nc.vector, "v")]
                qrot = [nc.sync, nc.gpsimd]
                idx = 0
                for dl in range(DPC):
                    for c in range(NB):
                        for b in range(B):
                            mi = (dl * NB + c) * B + b
                            eng, enm = engs[idx % 2]
                            o = wk.tile([128, 512], dt.float32, tag="fin", bufs=3, name="o")
                            if enm == "s":
                                nc.scalar.activation(o[:], mag2sb[mi][:], ACTF.Copy,
                                                     scale=ginv128[:])
                            else:
                                eng.tensor_scalar_mul(o[:], mag2sb[mi][:], ginv128[:])
                            qrot[idx % 2].dma_start(out_d[dl, c, b], o[:])
                            idx += 1

    nc.compile()
    return nc


_PROG_CACHE = {}


def _get_program():
    if "nc" not in _PROG_CACHE:
        _PROG_CACHE["nc"] = _build_program()
    return _PROG_CACHE["nc"]


# ---------------------------------------------------------------- cached runner
def _make_runner():
    """Build the jitted SPMD callable once; reuse across kernel() calls."""
    import jax
    from jax.sharding import Mesh, PartitionSpec
    from jax.experimental.shard_map import shard_map
    import concourse.mybir as mybir
    from concourse import bass2jax

    bass2jax.install_neuronx_cc_hook()
    nc = _get_program()

    partition_name = nc.partition_id_tensor.name if nc.partition_id_tensor else None
    in_names, out_names, out_avals, zero_shapes = [], [], [], []
    for alloc in nc.m.functions[0].allocations:
        if not isinstance(alloc, mybir.MemoryLocationSet):
            continue
        if not alloc.memorylocations:
            continue
        name = alloc.memorylocations[0].name
        if alloc.kind == "ExternalInput":
            if name != partition_name:
                in_names.append(name)
        elif alloc.kind == "ExternalOutput":
            out_names.append(name)
            shape = tuple(alloc.tensor_shape)
            dtype = mybir.dt.np(alloc.dtype)
            out_avals.append(jax.core.ShapedArray(shape, dtype))
            zero_shapes.append((shape, dtype))
    n_params = len(in_names)
    n_outs = len(out_avals)
    all_in_names = list(in_names) + list(out_names)
    if partition_name is not None:
        all_in_names.append(partition_name)
    donate = tuple(range(n_params, n_params + n_outs))

    def _body(*args):
        operands = list(args)
        if partition_name is not None:
            operands.append(bass2jax.partition_id_tensor())
        outs = bass2jax._bass_exec_p.bind(
            *operands,
            out_avals=tuple(out_avals),
            in_names=tuple(all_in_names),
            out_names=tuple(out_names),
            lowering_input_output_aliases=(),
            sim_require_finite=True,
            sim_require_nnan=True,
            nc=nc,
        )
        return tuple(outs)

    devices = jax.devices()[:NCORES]
    mesh = Mesh(np.asarray(devices), ("core",))
    in_specs = (PartitionSpec("core"),) * (n_params + n_outs)
    out_specs = (PartitionSpec("core"),) * n_outs
    sharded = jax.jit(
        shard_map(_body, mesh=mesh, in_specs=in_specs, out_specs=out_specs,
                  check_rep=False),
        donate_argnums=donate, keep_unused=True)

    def run(in_maps):
        concat_in = [
            np.concatenate([np.asarray(m[name]) for m in in_maps], axis=0)
            for name in in_names
        ]
        concat_zeros = [
            np.zeros((NCORES * s[0], *s[1:]), d) for (s, d) in zero_shapes
        ]
        out_arrs = sharded(*concat_in, *concat_zeros)
        return [
            {name: np.asarray(out_arrs[i]).reshape(NCORES, *out_avals[i].shape)[c]
             for i, name in enumerate(out_names)}
            for c in range(NCORES)
        ]

    return run


def _get_runner():
    if "run" not in _PROG_CACHE:
        _PROG_CACHE["run"] = _make_runner()
    return _PROG_CACHE["run"]


# ---------------------------------------------------------------- entry point
def _build_in_maps(img, Map, H, parameter):
    # img fields (b,c) -> [128, 6144]: col = (f*2+k)*256 + x
    imgt = img.transpose(0, 3, 1, 2).reshape(B * NB, 2, 128, 256)
    imgf = np.ascontiguousarray(imgt.transpose(2, 0, 1, 3).reshape(128, B * NB * 512))
    # Map -> per-core [4, 128, DPC*512]: col = (dl*2+rb)*256 + x
    mapt = Map.transpose(3, 0, 1, 2).reshape(ND, B, 2, 128, 256)
    ht = np.ascontiguousarray(H.reshape(16, 16).T)
    par = parameter.reshape(1, 1)
    in_maps = []
    for core in range(NCORES):
        mp = np.zeros((B, 128, DPC * 512), np.float32)
        msk = np.zeros((1, DPC), np.float32)
        for dl in range(DPC):
            d = core * DPC + dl
            if d < ND:
                fld = mapt[d].transpose(0, 2, 1, 3).reshape(B, 128, 512)
                mp[:, :, dl * 512:(dl + 1) * 512] = fld
                msk[0, dl] = 1.0
        in_maps.append({
            "imgf": imgf, "mapf": mp, "ht": ht, "param": par, "mask": msk,
        })
    return in_maps


def kernel(img, Map, H, parameter):
    img = np.ascontiguousarray(np.asarray(img, np.float32))
    Map = np.ascontiguousarray(np.asarray(Map, np.float32))
    H = np.asarray(H, np.float32)
    parameter = np.asarray(parameter, np.float32)

    try:
        run = _get_runner()
    except Exception:
        run = None

    in_maps = _build_in_maps(img, Map, H, parameter)

    if run is not None:
        try:
            results = run(in_maps)
        except Exception:
            run = None
    if run is None:
        from concourse.bass_utils import run_bass_kernel_spmd
        rr = run_bass_kernel_spmd(_get_program(), in_maps,
                                  core_ids=list(range(NCORES)))
        results = rr.results

    out = np.empty((B, 256, 256, NB * ND), np.float32)
    for core in range(NCORES):
        rec = results[core]["out_recov"]            # [DPC, NB, B, 256, 256]
        for dl in range(DPC):
            d = core * DPC + dl
            if d >= ND:
                continue
            for c in range(NB):
                for b in range(B):
                    out[b, :, :, c * ND + d] = (
                        rec[dl, c, b].reshape(128, 2, 256)
                        .transpose(1, 0, 2).reshape(256, 256))
    return out
